# revision 1
# baseline (speedup 1.0000x reference)
"""AnswerDecoder (LSTM decoder w/ visual attention) on 8 TRN2 NeuronCores.

Strategy: pure data-parallel over batch (8 batches/core), zero collectives.
Host does layout prep only (transposes/concat/embedding gather = data movement);
all FLOPs run on device.

Device algorithm per core (B_l = 8 local batches):
  pre:   h0 = q @ Wh.T ; c0 = q @ Wc.T ; o0 = g @ Wg2o.T + b
         gates_y[(t,b)] = y_emb[t,b] @ Wy.T + (b_ih+b_hh)    (teacher-forced part)
         apT[h, (b,r)] = attn_proj = L @ W_attn.T            (K=H layout for e-matmul)
         pbd[(b,r), j] = P = L @ W_u[:, :F].T                (K=(b,r) layout for v-matmul)
  loop t = 0..31 (gates PSUM for step t produced in iter t-1):
         LSTM cell -> h, c ; transpose h -> hT
         E[b, (b',r)] = hT.T @ apT ; masked softmax -> att (block-diagonal)
         transpose att -> abd[(b,r), b]
         vo = h @ Wuh.T + b_u + abd.T @ pbd ; o = tanh(vo)
         transpose o -> O_T slot t+1 (f32 for recurrence, bf16 for vocab)
         gates(t+1) = gy[t+1] + [o;h] @ Wr.T   (via PSUM accumulation)
  post:  logits[(t,b), v] = O_Tb.T @ Wv.T + b_vocab  (bf16 weights, f32 accum)
"""

import numpy as np

B, T, R = 64, 32, 49
LOCAL, QVEC, EMB, HID, VOCAB = 1024, 512, 256, 512, 10000
START_IDX = 1
NCORES = 8
BL = B // NCORES        # 8 batches per core
ROWS = T * BL           # 256 output rows per core, t-major (row = t*BL + b)
VCHUNK = 1250           # vocab dma chunk (mm chunks: 2x512 + 226)

USE_F32R = True


def _perm_gates():
    # reference gate order [i, f, g, o] -> device order [i, f, o, g]
    p = np.concatenate([np.arange(0, 512), np.arange(512, 1024),
                        np.arange(1536, 2048), np.arange(1024, 1536)])
    return p


def _tf32(x):
    """Round fp32 -> tf32 (fp32r) with round-to-nearest-even, as the PE expects."""
    x = np.ascontiguousarray(x, np.float32)
    u = x.view(np.uint32).copy()
    lsb = (u >> np.uint32(13)) & np.uint32(1)
    u = (u + np.uint32(0xFFF) + lsb) & np.uint32(0xFFFFE000)
    return u.view(np.float32)


def prep_inputs(image_local_features, image_global_features, question_vectors,
                answers, emb, W_g2o, b_g2o, W_h, W_c, W_ih, W_hh, b_ih, b_hh,
                W_attn, W_u, b_u, W_vocab, b_vocab):
    """Host-side data layout prep. Returns list of per-core input dicts."""
    import ml_dtypes

    f32 = np.float32
    L = np.ascontiguousarray(image_local_features, dtype=f32)       # [B,R,F]
    g = np.ascontiguousarray(image_global_features, dtype=f32)      # [B,2F]
    q = np.ascontiguousarray(question_vectors, dtype=f32)           # [B,Q]
    ans = np.asarray(answers).astype(np.int64)                      # [B,T]
    emb = np.asarray(emb, dtype=f32)

    perm = _perm_gates()
    # shared (batch-independent) weights
    Wy = W_ih[perm, :EMB].astype(f32)                               # [2048,256]
    bias_g = (b_ih + b_hh)[perm].astype(f32)                        # [2048]
    wy_aug = np.concatenate([Wy.T, bias_g[None, :]], 0)             # [257,2048]
    wr = np.concatenate([W_ih[perm, EMB:EMB + HID], W_hh[perm]], 1).T  # [1024,2048]
    wr[HID:, :] *= 0.5        # h-part rows compensate h' = 2h
    wat = np.ascontiguousarray(0.5 * W_attn.astype(f32).T)          # [1024,512]
    wua = np.ascontiguousarray(W_u[:, :LOCAL].astype(f32).T)        # [1024,512]
    wuh = np.concatenate([0.5 * W_u[:, LOCAL:].astype(f32).T,
                          np.asarray(b_u, f32)[None, :]], 0)        # [513,512]
    # h state is kept as h' = 2h and c as c' = 2c (sigmoid-via-tanh trick:
    # sigmoid(x) = (1+tanh(x/2))/2, so every ACT op is Tanh/Exp and the ACT
    # engine never swaps its function table). Weights touching h are halved;
    # wh/wc doubled so h0' = 2*h0, c0' = 2*c0.
    wh = np.ascontiguousarray(2.0 * W_h.astype(f32).T)              # [512,512]
    wc = np.ascontiguousarray(2.0 * W_c.astype(f32).T)              # [512,512]
    wg = np.concatenate([W_g2o.astype(f32).T,
                         np.asarray(b_g2o, f32)[None, :]], 0)       # [2049,512]
    wv = np.concatenate([W_vocab.astype(f32).T,
                         np.asarray(b_vocab, f32)[None, :]], 0)     # [513,10000]
    wv = wv.astype(ml_dtypes.bfloat16)

    mask = np.full((BL, BL * R), -1e9, f32)
    for b in range(BL):
        mask[b, b * R:(b + 1) * R] = 0.0
    ident = np.eye(128, dtype=f32)
    ones = np.ones((1, 128), f32)
    onesb = np.ones((1, 128), ml_dtypes.bfloat16)

    # teacher-forced input embeddings: y_seq[t] = emb[ans[:, t-1]], y_seq[0]=emb[1]
    idx = np.concatenate([np.full((B, 1), START_IDX, np.int64), ans[:, :-1]], 1)
    y_emb = emb[idx]                                                # [B,T,EMB]

    in_maps = []
    for c in range(NCORES):
        s = slice(c * BL, (c + 1) * BL)
        # y_aug_T [257, 256]: rows t-major (t*BL+b), transposed, ones row
        ye = y_emb[s].transpose(1, 0, 2).reshape(ROWS, EMB)         # [(t,b), EMB]
        yT = np.concatenate([ye.T, np.ones((1, ROWS), f32)], 0)     # [257,256]
        lT = np.ascontiguousarray(L[s].reshape(BL * R, LOCAL).T)    # [1024,392]
        qT = np.ascontiguousarray(q[s].T)                           # [512,8]
        gT = np.concatenate([g[s].T, np.ones((1, BL), f32)], 0)     # [2049,8]
        rnd = _tf32 if USE_F32R else (lambda a: np.ascontiguousarray(a, np.float32))
        in_maps.append({
            "yT": rnd(yT),
            "wy": rnd(wy_aug),
            "wr": rnd(wr),
            "lT": rnd(lT),
            "wat": rnd(wat),
            "wua": rnd(wua),
            "wuh": rnd(wuh),
            "qT": rnd(qT),
            "wh": rnd(wh),
            "wc": rnd(wc),
            "gT": rnd(gT),
            "wg": rnd(wg),
            "wv": np.ascontiguousarray(wv),
            "mask": mask,
            "ident": ident,
            "identr": ident.copy(),
            "ones": rnd(ones),
            "onesb": onesb,
        })
    return in_maps


def build_nc():
    """Build the per-core Bass graph (identical on all 8 cores)."""
    from concourse import bacc, tile
    import concourse.mybir as mybir

    f32 = mybir.dt.float32
    f32r = mybir.dt.float32r
    bf16 = mybir.dt.bfloat16
    AF = mybir.ActivationFunctionType
    ALU = mybir.AluOpType

    nc = bacc.Bacc("TRN2", target_bir_lowering=False, debug=False,
                   num_devices=NCORES)

    def dparam(name, shape, dt=f32):
        return nc.dram_tensor(name, list(shape), dt, kind="ExternalInput").ap()

    fr = f32r if USE_F32R else f32
    yT_d = dparam("yT", [257, 256], fr)
    wy_d = dparam("wy", [257, 2048], fr)
    wr_d = dparam("wr", [1024, 2048], fr)
    lT_d = dparam("lT", [1024, BL * R], fr)
    wat_d = dparam("wat", [1024, 512], fr)
    wua_d = dparam("wua", [1024, 512], fr)
    wuh_d = dparam("wuh", [513, 512], fr)
    qT_d = dparam("qT", [512, BL], fr)
    wh_d = dparam("wh", [512, 512], fr)
    wc_d = dparam("wc", [512, 512], fr)
    gT_d = dparam("gT", [2049, BL], fr)
    wg_d = dparam("wg", [2049, 512], fr)
    wv_d = dparam("wv", [513, VOCAB], bf16)
    mask_d = dparam("mask", [BL, BL * R])
    id_d = dparam("ident", [128, 128])
    idr_d = dparam("identr", [128, 128], fr)
    ones_d = dparam("ones", [1, 128], fr)
    onesb_d = dparam("onesb", [1, 128], bf16)
    out_d = nc.dram_tensor("out", [ROWS, VOCAB], f32, kind="ExternalOutput").ap()

    def rr(ap):  # dtypes are carried by the tiles themselves now
        return ap

    BR = BL * R  # 392
    # (b,r) partition-tile sizes: 128,128,128,8
    brsz = [128, 128, 128, BR - 384]

    with tile.TileContext(nc) as tc:
        import contextlib
        stack = contextlib.ExitStack()
        with stack:
            pers = stack.enter_context(tc.tile_pool(name="pers", bufs=1))
            stb1 = stack.enter_context(tc.tile_pool(name="stb1", bufs=1))
            stb2 = stack.enter_context(tc.tile_pool(name="stb2", bufs=2))
            lpool = stack.enter_context(tc.tile_pool(name="lg", bufs=3))
            gyp = stack.enter_context(tc.tile_pool(name="gyp", bufs=2))
            gyd = stack.enter_context(tc.tile_pool(name="gyd", bufs=1, space="DRAM"))
            pg = stack.enter_context(tc.tile_pool(name="pg", bufs=1, space="PSUM"))
            pmm = stack.enter_context(tc.tile_pool(name="pmm", bufs=2, space="PSUM"))
            ptp = stack.enter_context(tc.tile_pool(name="ptp", bufs=2, space="PSUM"))

            # ---- persistent SBUF ----
            wr_sb = pers.tile([128, 8, 2048], fr)
            gy_dram = gyd.tile([ROWS, 2048], fr)
            apT_sb = pers.tile([128, 4, BR], fr)
            pbd_sb = pers.tile([128, 4, 512], fr)
            ot_sb = pers.tile([128, 4, T + 1, BL], fr)
            otb_sb = pers.tile([128, 4, T, BL], bf16)
            wuh_sb = pers.tile([128, 5, 512], fr)
            mask_sb = pers.tile([BL, BR], f32)
            id_sb = pers.tile([128, 128], f32)
            ones_sb = pers.tile([1, 128], fr)
            idr_sb = pers.tile([128, 128], fr)
            onesb_sb = pers.tile([1, 128], bf16)
            c_sb = pers.tile([BL, 512], f32)

            dma = nc.sync.dma_start
            dma(out=mask_sb[:, :], in_=mask_d[:, :])
            dma(out=id_sb[:, :], in_=id_d[:, :])
            dma(out=ones_sb[:, :], in_=ones_d[:, :])
            dma(out=idr_sb[:, :], in_=idr_d[:, :])
            dma(out=onesb_sb[:, :], in_=onesb_d[:, :])

            id8 = id_sb[0:8, 0:8]
            id8r = idr_sb[0:8, 0:8]
            mm = nc.tensor.matmul

            def load_wr_wuh():
                for k in range(8):
                    dma(out=wr_sb[:, k, :], in_=wr_d[k * 128:(k + 1) * 128, :])
                for k in range(4):
                    dma(out=wuh_sb[:, k, :], in_=wuh_d[k * 128:(k + 1) * 128, :])
                dma(out=wuh_sb[0:1, 4, :], in_=wuh_d[512:513, :])

            # ---- precompute: h0/c0 ----
            hT = None
            with tc.tile_pool(name="preA1", bufs=1) as preA1:
                q_sb = preA1.tile([128, 4, BL], fr)
                wh_sb = preA1.tile([128, 4, 512], fr)
                wc_sb = preA1.tile([128, 4, 512], fr)
                hc0_sb = preA1.tile([BL, 2, 512], f32)
                for k in range(4):
                    dma(out=q_sb[:, k, :], in_=qT_d[k * 128:(k + 1) * 128, :])
                    dma(out=wh_sb[:, k, :], in_=wh_d[k * 128:(k + 1) * 128, :])
                    dma(out=wc_sb[:, k, :], in_=wc_d[k * 128:(k + 1) * 128, :])
                for i, w_sb in enumerate([wh_sb, wc_sb]):
                    ps = pmm.tile([BL, 512], f32, tag="mm")
                    for k in range(4):
                        mm(ps[:, :], rr(q_sb[:, k, :]), rr(w_sb[:, k, :]),
                           start=(k == 0), stop=(k == 3))
                    nc.scalar.copy(hc0_sb[:, i, :], ps[:, :])
                nc.vector.tensor_copy(c_sb[:, :], hc0_sb[:, 1, :])
                tp = ptp.tile([128, 4, BL], f32, tag="tp")
                for j in range(4):
                    nc.tensor.transpose(tp[:, j, :],
                                        hc0_sb[:, 0, j * 128:(j + 1) * 128], id8)
                hT = stb2.tile([128, 4, BL], fr, tag="hT")
                nc.vector.tensor_copy(hT[:, :, :], tp[:, :, :])

            load_wr_wuh()

            # ---- precompute: o0 ----
            with tc.tile_pool(name="preA2", bufs=1) as preA2:
                g_sb = preA2.tile([128, 17, BL], fr)
                wg_sb = preA2.tile([128, 17, 512], fr)
                for k in range(16):
                    dma(out=g_sb[:, k, :], in_=gT_d[k * 128:(k + 1) * 128, :])
                    dma(out=wg_sb[:, k, :], in_=wg_d[k * 128:(k + 1) * 128, :])
                dma(out=g_sb[0:1, 16, :], in_=gT_d[2048:2049, :])
                dma(out=wg_sb[0:1, 16, :], in_=wg_d[2048:2049, :])
                ps = pmm.tile([BL, 512], f32, tag="mm")
                for k in range(17):
                    lhsT = g_sb[0:1, 16, :] if k == 16 else g_sb[:, k, :]
                    rhs = wg_sb[0:1, 16, :] if k == 16 else wg_sb[:, k, :]
                    mm(ps[:, :], rr(lhsT), rr(rhs), start=(k == 0), stop=(k == 16))
                o0_sb = stb2.tile([BL, 512], f32, tag="o")
                nc.scalar.copy(o0_sb[:, :], ps[:, :])
                tp = ptp.tile([128, 4, BL], f32, tag="tp")
                for j in range(4):
                    nc.tensor.transpose(tp[:, j, :], o0_sb[:, j * 128:(j + 1) * 128],
                                        id8)
                nc.vector.tensor_copy(ot_sb[:, :, 0, :], tp[:, :, :])

            # ---- precompute: gates_y -> DRAM scratch ----
            # (per-step slices are re-loaded at partition base 0, a matmul
            #  operand requirement)
            with tc.tile_pool(name="preB", bufs=1) as preB:
                y_sb = preB.tile([128, 3, 256], fr)
                wy_sb = preB.tile([128, 3, 2048], fr)
                for k in range(2):
                    dma(out=y_sb[:, k, :], in_=yT_d[k * 128:(k + 1) * 128, :])
                    dma(out=wy_sb[:, k, :], in_=wy_d[k * 128:(k + 1) * 128, :])
                dma(out=y_sb[0:1, 2, :], in_=yT_d[256:257, :])
                dma(out=wy_sb[0:1, 2, :], in_=wy_d[256:257, :])
                for m in range(2):
                    for j in range(4):
                        ps = pmm.tile([128, 512], f32, tag="mm")
                        for k in range(3):
                            lhsT = (y_sb[0:1, 2, m * 128:(m + 1) * 128] if k == 2
                                    else y_sb[:, k, m * 128:(m + 1) * 128])
                            rhs = (wy_sb[0:1, 2, j * 512:(j + 1) * 512] if k == 2
                                   else wy_sb[:, k, j * 512:(j + 1) * 512])
                            mm(ps[:, :], rr(lhsT), rr(rhs), start=(k == 0),
                               stop=(k == 2))
                        lg = lpool.tile([128, 512], fr, tag="lg")
                        nc.vector.tensor_copy(lg[:, :], ps[:, :])
                        dma(out=gy_dram[m * 128:(m + 1) * 128,
                                        j * 512:(j + 1) * 512],
                            in_=lg[:, :])

            # ---- gates(0) (overlaps preC DMAs) ----
            gyt = gyp.tile([BL, 2048], fr, tag="gyt")
            dma(out=gyt[:, :], in_=gy_dram[0:BL, :])
            g_ps = [pg.tile([BL, 512], f32, name=f"gps{j}", tag=f"g{j}")
                    for j in range(4)]
            for j in range(4):
                js = slice(j * 512, (j + 1) * 512)
                mm(g_ps[j][:, :], rr(id8r), rr(gyt[0:8, js]), start=True,
                   stop=False)
                for k in range(4):
                    mm(g_ps[j][:, :], rr(hT[:, k, :]), rr(wr_sb[:, 4 + k, js]),
                       start=False, stop=False)
                for k in range(4):
                    mm(g_ps[j][:, :], rr(ot_sb[:, k, 0, :]), rr(wr_sb[:, k, js]),
                       start=False, stop=(k == 3))

            # ---- precompute: apT + pbd ----
            with tc.tile_pool(name="preC", bufs=1) as preC:
                lT_sb = preC.tile([128, 8, BR], fr)
                for k in range(8):
                    dma(out=lT_sb[:, k, :], in_=lT_d[k * 128:(k + 1) * 128, :])
                with tc.tile_pool(name="preC1", bufs=1) as preC1:
                    wat_sb = preC1.tile([128, 8, 512], fr)
                    for k in range(8):
                        dma(out=wat_sb[:, k, :], in_=wat_d[k * 128:(k + 1) * 128, :])
                    for hk in range(4):
                        ps = pmm.tile([128, BR], f32, tag="mm")
                        for k in range(8):
                            mm(ps[:, :], rr(wat_sb[:, k, hk * 128:(hk + 1) * 128]),
                               rr(lT_sb[:, k, :]), start=(k == 0), stop=(k == 7))
                        nc.vector.tensor_copy(apT_sb[:, hk, :], ps[:, :])
                with tc.tile_pool(name="preC2", bufs=1) as preC2:
                    wua_sb = preC2.tile([128, 8, 512], fr)
                    for k in range(8):
                        dma(out=wua_sb[:, k, :], in_=wua_d[k * 128:(k + 1) * 128, :])
                    for mt in range(4):
                        sz = brsz[mt]
                        ps = pmm.tile([128, 512], f32, tag="mm")
                        for k in range(8):
                            mm(ps[0:sz, :], rr(lT_sb[:, k, mt * 128:mt * 128 + sz]),
                               rr(wua_sb[:, k, :]), start=(k == 0), stop=(k == 7))
                        nc.vector.tensor_copy(pbd_sb[0:sz, mt, :], ps[0:sz, :])

            # ---- vocab weight prefetch (overlaps the whole recurrence) ----
            wvp = stack.enter_context(tc.tile_pool(name="wvp", bufs=3))
            nmm = [(0, 512), (512, 512), (1024, 226)]
            wv_tiles = {}

            def fetch_wv(vc):
                wv_t = wvp.tile([128, 5, VCHUNK], bf16, tag="wv")
                vb = vc * VCHUNK
                for k in range(4):
                    dma(out=wv_t[:, k, :],
                        in_=wv_d[k * 128:(k + 1) * 128, vb:vb + VCHUNK])
                dma(out=wv_t[0:1, 4, :], in_=wv_d[512:513, vb:vb + VCHUNK])
                wv_tiles[vc] = wv_t

            fetch_wv(0)
            fetch_wv(1)

            # ---- recurrence ----
            # gates chunk order [i, f, og, g]; matmuls emit chunk g first so
            # tanh(g) starts while the i/f/og chunks still accumulate.
            CHORD = [3, 0, 1, 2]
            ADD, MULT = ALU.add, ALU.mult
            stt = nc.vector.scalar_tensor_tensor
            for t in range(T):
                # LSTM cell, all-Tanh form. State: c_sb = 2c, h tile = 2h.
                tg = stb1.tile([BL, 512], f32, tag="tg")
                nc.scalar.activation(tg[:, :], g_ps[3][:, :], AF.Tanh)
                ti = stb1.tile([BL, 512], f32, tag="ti")
                nc.scalar.activation(ti[:, :], g_ps[0][:, :], AF.Tanh, scale=0.5)
                tf_ = stb1.tile([BL, 512], f32, tag="tf")
                nc.scalar.activation(tf_[:, :], g_ps[1][:, :], AF.Tanh, scale=0.5)
                tog = stb1.tile([BL, 512], f32, tag="tog")
                nc.scalar.activation(tog[:, :], g_ps[2][:, :], AF.Tanh, scale=0.5)
                t1 = stb1.tile([BL, 512], f32, tag="t1")
                stt(t1[:, :], ti[:, :], 1.0, tg[:, :], op0=ADD, op1=MULT)
                m2 = stb1.tile([BL, 512], f32, tag="m2")
                stt(m2[:, :], tf_[:, :], 1.0, c_sb[:, :], op0=ADD, op1=MULT)
                stt(c_sb[:, :], m2[:, :], 0.5, t1[:, :], op0=MULT, op1=ADD)
                tc2 = stb1.tile([BL, 512], f32, tag="tc2")
                nc.scalar.activation(tc2[:, :], c_sb[:, :], AF.Tanh, scale=0.5)
                h = stb2.tile([BL, 512], f32, tag="h")
                stt(h[:, :], tog[:, :], 1.0, tc2[:, :], op0=ADD, op1=MULT)

                tp = ptp.tile([128, 4, BL], f32, tag="tp")
                for j in range(4):
                    nc.tensor.transpose(tp[:, j, :], h[:, j * 128:(j + 1) * 128], id8)
                hT = stb2.tile([128, 4, BL], fr, tag="hT")
                nc.vector.tensor_copy(hT[:, :, :], tp[:, :, :])

                # e matmul -> E_full [8, 392]
                E = pmm.tile([BL, BR], f32, tag="mm")
                for k in range(4):
                    mm(E[:, :], rr(hT[:, k, :]), rr(apT_sb[:, k, :]),
                       start=(k == 0), stop=(k == 3))

                # gates(t+1): gy + h-part (runs on PE during softmax)
                if t < T - 1:
                    gyt = gyp.tile([BL, 2048], fr, tag="gyt")
                    dma(out=gyt[:, :], in_=gy_dram[(t + 1) * BL:(t + 2) * BL, :])
                    g_next = [pg.tile([BL, 512], f32, name=f"gnx{j}", tag=f"g{j}")
                              for j in range(4)]
                    for j in CHORD:
                        js = slice(j * 512, (j + 1) * 512)
                        mm(g_next[j][:, :], rr(id8r), rr(gyt[0:8, js]),
                           start=True, stop=False, skip_group_check=True)
                        for k in range(4):
                            mm(g_next[j][:, :], rr(hT[:, k, :]), rr(wr_sb[:, 4 + k, js]),
                               start=False, stop=False, skip_group_check=True)

                # masked softmax over the diagonal blocks
                esb = stb2.tile([BL, BR], f32, tag="esb")
                nc.vector.tensor_add(esb[:, :], E[:, :], mask_sb[:, :])
                nmax = stb2.tile([BL, 1], f32, tag="nmax")
                nc.vector.tensor_reduce(nmax[:, :], esb[:, :],
                                        mybir.AxisListType.X, ALU.max, negate=True)
                expv = stb2.tile([BL, BR], f32, tag="expv")
                ssum = stb2.tile([BL, 1], f32, tag="ssum")
                nc.scalar.activation(expv[:, :], esb[:, :], AF.Exp,
                                     bias=nmax[:, :], scale=1.0,
                                     accum_out=ssum[:, :])
                rs = stb2.tile([BL, 1], f32, tag="rs")
                nc.vector.reciprocal(rs[:, :], ssum[:, :])
                att = stb2.tile([BL, BR], f32, tag="att")
                nc.vector.tensor_scalar_mul(att[:, :], expv[:, :], rs[:, :])

                # transpose att -> block-diagonal [(b,r), b]
                tpa = ptp.tile([128, 4, BL], f32, tag="tp")
                for j in range(4):
                    sz = brsz[j]
                    nc.tensor.transpose(tpa[0:sz, j, :], att[:, j * 128:j * 128 + sz],
                                        id8)
                abd = stb2.tile([128, 4, BL], fr, tag="abd")
                nc.vector.tensor_copy(abd[:, 0:3, :], tpa[:, 0:3, :])
                nc.vector.tensor_copy(abd[0:8, 3, :], tpa[0:8, 3, :])

                # vo = h @ Wuh.T + b_u + att-weighted P
                vo = pmm.tile([BL, 512], f32, tag="mm")
                for k in range(4):
                    mm(vo[:, :], rr(hT[:, k, :]), rr(wuh_sb[:, k, :]),
                       start=(k == 0), stop=False)
                mm(vo[:, :], rr(ones_sb[0:1, 0:8]), rr(wuh_sb[0:1, 4, :]),
                   start=False, stop=False)
                for j in range(4):
                    sz = brsz[j]
                    mm(vo[:, :], rr(abd[0:sz, j, :]), rr(pbd_sb[0:sz, j, :]),
                       start=False, stop=(j == 3))

                o_sb = stb2.tile([BL, 512], f32, tag="o")
                nc.scalar.activation(o_sb[:, :], vo[:, :], AF.Tanh)
                tpo = ptp.tile([128, 4, BL], f32, tag="tp")
                for j in range(4):
                    nc.tensor.transpose(tpo[:, j, :], o_sb[:, j * 128:(j + 1) * 128],
                                        id8)
                nc.vector.tensor_copy(ot_sb[:, :, t + 1, :], tpo[:, :, :])
                nc.vector.tensor_copy(otb_sb[:, :, t, :], tpo[:, :, :])

                # gates(t+1): o-part
                if t < T - 1:
                    for j in CHORD:
                        js = slice(j * 512, (j + 1) * 512)
                        for k in range(4):
                            mm(g_next[j][:, :], rr(ot_sb[:, k, t + 1, :]),
                               rr(wr_sb[:, k, js]), start=False, stop=(k == 3),
                               skip_group_check=True)
                    g_ps = g_next

            # ---- vocab projection ----
            for vc in range(VOCAB // VCHUNK):
                wv_t = wv_tiles.pop(vc)
                vb = vc * VCHUNK
                if vc + 2 < VOCAB // VCHUNK:
                    fetch_wv(vc + 2)
                for m in range(2):
                    for nb, nsz in nmm:
                        ps = pmm.tile([128, 512], f32, tag="mm")
                        ns = slice(nb, nb + nsz)
                        for k in range(4):
                            mm(ps[:, 0:nsz], otb_sb[:, k, m * 16:(m + 1) * 16, :],
                               wv_t[:, k, ns], start=(k == 0), stop=False)
                        mm(ps[:, 0:nsz], onesb_sb[0:1, :], wv_t[0:1, 4, ns],
                           start=False, stop=True)
                        lg = lpool.tile([128, 512], f32, tag="lg")
                        nc.vector.tensor_copy(lg[:, 0:nsz], ps[:, 0:nsz])
                        dma(out=out_d[m * 128:(m + 1) * 128,
                                      vb + nb:vb + nb + nsz],
                            in_=lg[:, 0:nsz])

    nc.compile()
    return nc


_STATE = {}


def kernel(**inputs):
    from concourse.bass_utils import run_bass_kernel_spmd

    in_maps = prep_inputs(**inputs)
    if "nc" not in _STATE:
        _STATE["nc"] = build_nc()
    nc = _STATE["nc"]
    res = run_bass_kernel_spmd(nc, in_maps, core_ids=list(range(NCORES)))
    full = np.empty((B, T, VOCAB), np.float32)
    for c in range(NCORES):
        full[c * BL:(c + 1) * BL] = (
            res.results[c]["out"].reshape(T, BL, VOCAB).transpose(1, 0, 2))
    return full



# revision 6
# speedup vs baseline: 1.5617x; 1.5617x over previous
"""AnswerDecoder (LSTM decoder w/ visual attention) on 8 TRN2 NeuronCores.

Strategy: pure data-parallel over batch (8 batches/core), zero collectives.

v2 design notes (all relative to the measured v1 trace: 787us, PE-bound):
- "Quad" layout: partition 32j+b holds (hidden-slice j, batch b). All LSTM
  cell elementwise work runs as single [104, N] instructions instead of 4-8
  [8, N] ones (ACT/DVE cost is free-dim-dominated).
- 4-way PE column tiling: the four quad groups' matmuls use tile_position
  (0, 32j) and stream concurrently (measured 82.5 ns/MM for N=512 bf16 vs
  231.8 serial). Same-bank disjoint-partition accumulation verified OK.
- bf16 weights/activations everywhere on the PE (f32r at N<256 runs at 1/4
  rate; bf16 is 1 cycle/row always). c-state and PSUM stay f32.
- One batched PE transpose per h/att/o per step ([104,128] -> [128,128] via
  zero-padded identity) instead of 4 narrow transposes each.
- Softmax without max-subtraction (|e| << 80 so fp32 exp is safe); the
  block-diagonal mask is folded into the e-matmul as a K=8 identity wave;
  row sums come free from the Exp accumulator; normalization happens on the
  exp output before transposing.
- W_u bias enters via a ones-row in abd x b_u row in pbd; vocab bias is
  added on the host.
- Vocab projection: M-tile 0 (steps 0-15) is interleaved into PE idle slots
  of steps 16-31; only M-tile 1 runs after the loop. Output is bf16
  (upcast on host).
"""

import numpy as np

B, T, R = 64, 32, 49
LOCAL, QVEC, EMB, HID, VOCAB = 1024, 512, 256, 512, 10000
START_IDX = 1
NCORES = 8
BL = B // NCORES        # 8 batches per core
ROWS = T * BL           # 256 output rows per core, t-major (row = t*BL + b)
NEG = -60000.0          # mask value; exp(NEG + e) underflows to 0 in fp32


def _quad_perm_scale():
    """Device gate-column order: group j, gate [i,f,og,g], offset f.
    Returns (perm, scale): device col -> ref 4H row, and the 0.5 tanh-half
    scaling for i/f/og."""
    # ref row ranges: i 0:512, f 512:1024, g 1024:1536, o 1536:2048
    base = {0: 0, 1: 512, 2: 1536, 3: 1024}      # device gate idx -> ref base
    perm = np.empty(2048, np.int64)
    scale = np.empty(2048, np.float32)
    for j in range(4):
        for g in range(4):
            cols = slice(j * 512 + g * 128, j * 512 + g * 128 + 128)
            perm[cols] = base[g] + 128 * j + np.arange(128)
            scale[cols] = 0.5 if g < 3 else 1.0
    return perm, scale


def prep_inputs(image_local_features, image_global_features, question_vectors,
                answers, emb, W_g2o, b_g2o, W_h, W_c, W_ih, W_hh, b_ih, b_hh,
                W_attn, W_u, b_u, W_vocab, b_vocab):
    """Host-side data layout prep. Returns list of per-core input dicts."""
    import ml_dtypes
    bf16 = ml_dtypes.bfloat16
    f32 = np.float32

    L = np.asarray(image_local_features, f32)                   # [B,R,F]
    g = np.asarray(image_global_features, f32)                  # [B,2F]
    q = np.asarray(question_vectors, f32)                       # [B,Q]
    ans = np.asarray(answers).astype(np.int64)                  # [B,T]
    emb = np.asarray(emb, f32)

    perm, qscale = _quad_perm_scale()
    # recurrent weights: K = [o (512); h (512)], h-part halved (h' = 2h)
    W_cat = np.concatenate([W_ih[:, EMB:EMB + HID], 0.5 * np.asarray(W_hh, f32)],
                           axis=1)                              # [2048, 1024]
    wrq = (W_cat.T[:, perm] * qscale[None, :]).astype(bf16)     # [1024, 2048]
    wy_full = np.concatenate([np.asarray(W_ih, f32)[:, :EMB].T,
                              (np.asarray(b_ih, f32) + np.asarray(b_hh, f32))[None, :]],
                             axis=0)                            # [257, 2048ref]
    wyq = (wy_full[:, perm] * qscale[None, :]).astype(bf16)     # [257, 2048]

    whq = (2.0 * np.asarray(W_h, f32).T).astype(bf16)           # [512, 512]
    wcq = (2.0 * np.asarray(W_c, f32).T).astype(bf16)           # [512, 512]
    wgq = np.concatenate([np.asarray(W_g2o, f32).T,
                          np.asarray(b_g2o, f32)[None, :]], 0).astype(bf16)  # [2049,512]
    watq = (0.5 * np.asarray(W_attn, f32).T).astype(bf16)       # [1024, 512]
    wuaq = np.ascontiguousarray(np.asarray(W_u, f32)[:, :LOCAL].T).astype(bf16)
    wuhq = (0.5 * np.asarray(W_u, f32)[:, LOCAL:].T).astype(bf16)  # [512, 512]
    buq = np.asarray(b_u, f32)[None, :].astype(bf16)            # [1, 512]
    wv = np.ascontiguousarray(np.asarray(W_vocab, f32).T).astype(bf16)  # [512,10000]

    # col 2R is an epsilon column (-55 -> exp ~ 1.3e-24): keeps every row's
    # exp-sum nonzero so 1/ssum stays finite on fully-masked (off-diagonal)
    # rows; abd only consumes cols 0:2R so it never reaches vo.
    maskq = np.full((BL, 4, 2 * R + 1), NEG, f32)
    maskq[:, :, 2 * R] = -55.0
    for j in range(4):
        maskq[2 * j, j, 0:R] = 0.0
        maskq[2 * j + 1, j, R:2 * R] = 0.0
    maskq = maskq.astype(bf16)
    idb = np.eye(128, dtype=f32).astype(bf16)
    ones8 = np.ones((1, BL), f32).astype(bf16)

    # teacher-forced input embeddings: y_seq[t] = emb[ans[:, t-1]], y_seq[0]=emb[1]
    idx = np.concatenate([np.full((B, 1), START_IDX, np.int64), ans[:, :-1]], 1)
    y_emb = emb[idx]                                            # [B,T,EMB]

    shared = {
        "wyq": wyq, "wrq": wrq, "whq": whq, "wcq": wcq, "wgq": wgq,
        "watq": watq, "wuaq": wuaq, "wuhq": wuhq, "buq": buq, "wv": wv,
        "maskq": maskq, "idb": idb, "ones8": ones8,
    }
    in_maps = []
    for c in range(NCORES):
        s = slice(c * BL, (c + 1) * BL)
        ye = y_emb[s].transpose(1, 0, 2).reshape(ROWS, EMB)     # [(t,b), EMB]
        yT = np.concatenate([ye.T, np.ones((1, ROWS), f32)], 0).astype(bf16)
        lT = np.ascontiguousarray(L[s].reshape(BL * R, LOCAL).T).astype(bf16)
        qT = np.ascontiguousarray(q[s].T).astype(bf16)          # [512,8]
        gT = np.concatenate([g[s].T, np.ones((1, BL), f32)], 0).astype(bf16)
        d = {"yT": yT, "lT": lT, "qT": qT, "gT": gT}
        d.update(shared)
        in_maps.append(d)
    return in_maps


def build_nc():
    """Build the per-core Bass graph (identical on all 8 cores)."""
    from concourse import bacc, tile
    import concourse.mybir as mybir

    f32 = mybir.dt.float32
    bf16 = mybir.dt.bfloat16
    AF = mybir.ActivationFunctionType
    ALU = mybir.AluOpType

    nc = bacc.Bacc("TRN2", target_bir_lowering=False, debug=False,
                   num_devices=NCORES)

    def dparam(name, shape, dt=bf16):
        return nc.dram_tensor(name, list(shape), dt, kind="ExternalInput").ap()

    yT_d = dparam("yT", [257, 256])
    wyq_d = dparam("wyq", [257, 2048])
    wrq_d = dparam("wrq", [1024, 2048])
    lT_d = dparam("lT", [1024, BL * R])
    watq_d = dparam("watq", [1024, 512])
    wuaq_d = dparam("wuaq", [1024, 512])
    wuhq_d = dparam("wuhq", [512, 512])
    qT_d = dparam("qT", [512, BL])
    whq_d = dparam("whq", [512, 512])
    wcq_d = dparam("wcq", [512, 512])
    gT_d = dparam("gT", [2049, BL])
    wgq_d = dparam("wgq", [2049, 512])
    buq_d = dparam("buq", [1, 512])
    wv_d = dparam("wv", [512, VOCAB])
    mask_d = dparam("maskq", [BL, 4, 2 * R + 1])
    idb_d = dparam("idb", [128, 128])
    ones8_d = dparam("ones8", [1, BL])
    out_d = nc.dram_tensor("out", [ROWS, VOCAB], bf16, kind="ExternalOutput").ap()

    mm = nc.tensor.matmul
    ADD, MULT = ALU.add, ALU.mult
    stt = nc.vector.scalar_tensor_tensor
    vcp = nc.vector.tensor_copy
    P104 = 104  # 3*32 + 8: spans all four quad groups

    with tile.TileContext(nc) as tc:
        import contextlib
        stack = contextlib.ExitStack()
        with stack:
            pers = stack.enter_context(tc.tile_pool(name="pers", bufs=1))
            stb = stack.enter_context(tc.tile_pool(name="stb", bufs=2))
            gyp = stack.enter_context(tc.tile_pool(name="gyp", bufs=2))
            gyd = stack.enter_context(tc.tile_pool(name="gyd", bufs=1, space="DRAM"))
            pg = stack.enter_context(tc.tile_pool(name="pg", bufs=2, space="PSUM"))
            ptp = stack.enter_context(tc.tile_pool(name="ptp", bufs=2, space="PSUM"))

            dma = nc.sync.dma_start

            # ---- persistent SBUF ----
            wr_sb = pers.tile([128, 8, 2048], bf16)
            wuh_sb = pers.tile([128, 4, 512], bf16)
            apT_sb = pers.tile([128, 4, BL * R], bf16)
            pbd_sb = pers.tile([128, 4, 512], bf16)
            otb_sb = pers.tile([128, 4, T + 1, BL], bf16)
            wv_sb = pers.tile([128, 4, VOCAB], bf16)
            cq_sb = pers.tile([128, 128], f32)
            abd_sb = pers.tile([128, 4, 32], bf16)
            mask_sb = pers.tile([BL, 4, 2 * R + 1], bf16)
            idb_sb = pers.tile([128, 128], bf16)
            gy_dram = gyd.tile([ROWS, 2048], bf16)

            dma(out=mask_sb[:, :, :], in_=mask_d[:, :, :])
            dma(out=idb_sb[:, :], in_=idb_d[:, :])
            dma(out=abd_sb[2 * R:2 * R + 1, 0, 0:BL], in_=ones8_d[:, :])
            dma(out=pbd_sb[2 * R:2 * R + 1, 0, :], in_=buq_d[:, :])
            for k in range(4):
                dma(out=wuh_sb[:, k, :], in_=wuhq_d[128 * k:128 * (k + 1), :])
            for k in range(8):
                dma(out=wr_sb[:, k, :], in_=wrq_d[128 * k:128 * (k + 1), :])

            id8 = idb_sb[0:8, 0:8]
            idT = idb_sb[0:P104, 0:128]   # zero-padded transpose identity

            def quad_mm(out_t, lhsT, rhs, j, start, stop):
                mm(out_t[32 * j:32 * j + BL, :], lhsT, rhs, start=start,
                   stop=stop, skip_group_check=True, tile_position=(0, 32 * j))

            def transpose104(in_ap, src_pool=None):
                """[104, F] -> ptp tile [128, 4, 32] (cols 32k+b valid)."""
                tp = ptp.tile([128, 4, 32], bf16, name="tp", tag="tp")
                nP = in_ap.shape[0]
                nc.tensor.transpose(tp[0:in_ap.shape[1], :, :], in_ap,
                                    idb_sb[0:nP, 0:128])
                return tp

            # ---- preamble ----
            hT_sb = None
            with tc.tile_pool(name="pre", bufs=1) as pre, \
                 tc.tile_pool(name="pmm", bufs=2, space="PSUM") as pmm:
                # h0 / c0 (quad)
                q_sb = pre.tile([128, 4, BL], bf16)
                wh_sb = pre.tile([128, 4, 512], bf16)
                wc_sb = pre.tile([128, 4, 512], bf16)
                for k in range(4):
                    dma(out=q_sb[:, k, :], in_=qT_d[128 * k:128 * (k + 1), :])
                    dma(out=wh_sb[:, k, :], in_=whq_d[128 * k:128 * (k + 1), :])
                    dma(out=wc_sb[:, k, :], in_=wcq_d[128 * k:128 * (k + 1), :])
                hq0 = pmm.tile([128, 128], f32, name="hq0", tag="mmq")
                cq0 = pmm.tile([128, 128], f32, name="cq0", tag="mmq")
                for dst, w_sb in ((hq0, wh_sb), (cq0, wc_sb)):
                    for k in range(4):
                        for j in range(4):
                            quad_mm(dst, q_sb[:, k, :],
                                    w_sb[:, k, 128 * j:128 * (j + 1)], j,
                                    start=(k == 0), stop=(k == 3))
                h_sb = stb.tile([128, 128], bf16, name="h", tag="h")
                vcp(h_sb[0:P104, :], hq0[0:P104, :])
                vcp(cq_sb[0:P104, :], cq0[0:P104, :])
                tp = transpose104(h_sb[0:P104, :])
                hT_sb = stb.tile([128, 4, 32], bf16, name="hT", tag="hT")
                vcp(hT_sb[:, :, 0:BL], tp[:, :, 0:BL])

                # o0 (plain [8,512] then 4 narrow transposes into otb slot 0)
                g_sb = pre.tile([128, 17, BL], bf16)
                wg_sb = pre.tile([128, 17, 512], bf16)
                for k in range(16):
                    dma(out=g_sb[:, k, :], in_=gT_d[128 * k:128 * (k + 1), :])
                    dma(out=wg_sb[:, k, :], in_=wgq_d[128 * k:128 * (k + 1), :])
                dma(out=g_sb[0:1, 16, :], in_=gT_d[2048:2049, :])
                dma(out=wg_sb[0:1, 16, :], in_=wgq_d[2048:2049, :])
                o0ps = pmm.tile([BL, 512], f32, name="o0ps", tag="mmq")
                for k in range(17):
                    lhsT = g_sb[0:1, 16, :] if k == 16 else g_sb[:, k, :]
                    rhs = wg_sb[0:1, 16, :] if k == 16 else wg_sb[:, k, :]
                    mm(o0ps[:, :], lhsT, rhs, start=(k == 0), stop=(k == 16))
                o0_sb = stb.tile([BL, 512], bf16, name="o0", tag="o0")
                vcp(o0_sb[:, :], o0ps[:, :])
                tp0 = ptp.tile([128, 4, 32], bf16, name="tp0", tag="tp")
                for k in range(4):
                    nc.tensor.transpose(tp0[:, k, :],
                                        o0_sb[:, 128 * k:128 * (k + 1)],
                                        idb_sb[0:8, 0:32])
                vcp(otb_sb[:, :, 0, :], tp0[:, :, 0:BL])

                # gates_y -> DRAM scratch (bf16, quad-permuted cols)
                y_sb = pre.tile([128, 3, 256], bf16)
                wy_sb = pre.tile([128, 3, 2048], bf16)
                for k in range(2):
                    dma(out=y_sb[:, k, :], in_=yT_d[128 * k:128 * (k + 1), :])
                    dma(out=wy_sb[:, k, :], in_=wyq_d[128 * k:128 * (k + 1), :])
                dma(out=y_sb[0:1, 2, :], in_=yT_d[256:257, :])
                dma(out=wy_sb[0:1, 2, :], in_=wyq_d[256:257, :])
                for m in range(2):
                    for nb in range(4):
                        ps = pmm.tile([128, 512], f32, name="gyps", tag="mmq")
                        ns = slice(512 * nb, 512 * (nb + 1))
                        for k in range(3):
                            lhsT = (y_sb[0:1, 2, 128 * m:128 * (m + 1)] if k == 2
                                    else y_sb[:, k, 128 * m:128 * (m + 1)])
                            rhs = (wy_sb[0:1, 2, ns] if k == 2
                                   else wy_sb[:, k, ns])
                            mm(ps[:, :], lhsT, rhs, start=(k == 0), stop=(k == 2))
                        lg = stb.tile([128, 512], bf16, name="lg", tag="lg")
                        vcp(lg[:, :], ps[:, :])
                        dma(out=gy_dram[128 * m:128 * (m + 1), ns], in_=lg[:, :])

                # apT + pbd (shared lT)
                lT_sb = pre.tile([128, 8, BL * R], bf16)
                for k in range(8):
                    dma(out=lT_sb[:, k, :], in_=lT_d[128 * k:128 * (k + 1), :])
                wat_sb = pre.tile([128, 8, 512], bf16)
                for k in range(8):
                    dma(out=wat_sb[:, k, :], in_=watq_d[128 * k:128 * (k + 1), :])
                for hk in range(4):
                    ps = pmm.tile([128, BL * R], f32, name="apps", tag="mmq")
                    for k in range(8):
                        mm(ps[:, :], wat_sb[:, k, 128 * hk:128 * (hk + 1)],
                           lT_sb[:, k, :], start=(k == 0), stop=(k == 7))
                    vcp(apT_sb[:, hk, :], ps[:, :])
                wua_sb = pre.tile([128, 8, 512], bf16)
                for k in range(8):
                    dma(out=wua_sb[:, k, :], in_=wuaq_d[128 * k:128 * (k + 1), :])
                for mt in range(4):
                    ps = pmm.tile([128, 512], f32, name="pbps", tag="mmq")
                    for k in range(8):
                        mm(ps[0:2 * R, :],
                           lT_sb[:, k, 2 * R * mt:2 * R * (mt + 1)],
                           wua_sb[:, k, :], start=(k == 0), stop=(k == 7))
                    vcp(pbd_sb[0:2 * R, mt, :], ps[0:2 * R, :])

                # gates(0)
                gytq = gyp.tile([BL, 2048], bf16, name="gytq", tag="gytq")
                dma(out=gytq[:, :], in_=gy_dram[0:BL, :])
                gq = pg.tile([128, 512], f32, name="gq", tag="gq")
                for j in range(4):
                    quad_mm(gq, id8, gytq[0:8, 512 * j:512 * (j + 1)], j,
                            start=True, stop=False)
                for k in range(4):
                    for j in range(4):
                        quad_mm(gq, hT_sb[:, k, 0:BL],
                                wr_sb[:, 4 + k, 512 * j:512 * (j + 1)], j,
                                start=False, stop=False)
                for k in range(4):
                    for j in range(4):
                        quad_mm(gq, otb_sb[:, k, 0, :],
                                wr_sb[:, k, 512 * j:512 * (j + 1)], j,
                                start=False, stop=(k == 3))

            # ---- vocab weights resident in SBUF (streamed during loop) ----
            for vc in range(8):
                vs = slice(1250 * vc, 1250 * (vc + 1))
                for k in range(4):
                    dma(out=wv_sb[:, k, vs], in_=wv_d[128 * k:128 * (k + 1), vs])

            pe1 = stack.enter_context(tc.tile_pool(name="pe1", bufs=1, space="PSUM"))
            pv = stack.enter_context(tc.tile_pool(name="pv", bufs=1, space="PSUM"))
            pvoc = stack.enter_context(tc.tile_pool(name="pvoc", bufs=2, space="PSUM"))

            # vocab units: (m_tile, col_base, n_size); M-tile 0 interleaved
            nmm = [(0, 512), (512, 512), (1024, 226)]
            units = [(m, 1250 * vc + nb, nsz)
                     for m in range(2) for vc in range(8) for nb, nsz in nmm]

            def vocab_unit(m, cb, nsz):
                ps = pvoc.tile([128, 512], f32, name="vps", tag="vps")
                for k in range(4):
                    mm(ps[:, 0:nsz], otb_sb[:, k, 1 + 16 * m:17 + 16 * m, :],
                       wv_sb[:, k, cb:cb + nsz], start=(k == 0), stop=(k == 3),
                       skip_group_check=True)
                lg = stb.tile([128, 512], bf16, name="lg", tag="lg")
                vcp(lg[:, 0:nsz], ps[:, 0:nsz])
                dma(out=out_d[128 * m:128 * (m + 1), cb:cb + nsz],
                    in_=lg[:, 0:nsz])

            # ---- recurrence ----
            for t in range(T):
                if t < T - 1:
                    gytq = gyp.tile([BL, 2048], bf16, name="gytq", tag="gytq")
                    dma(out=gytq[:, :],
                        in_=gy_dram[BL * (t + 1):BL * (t + 2), :])

                # LSTM cell on quad layout; gq cols per group: [i|f|og|g]*128
                tgall = stb.tile([128, 512], bf16, name="tgall", tag="tgall")
                nc.scalar.activation(tgall[0:P104, :], gq[0:P104, :], AF.Tanh)
                ti = tgall[0:P104, 0:128]
                tf_ = tgall[0:P104, 128:256]
                tog = tgall[0:P104, 256:384]
                tg = tgall[0:P104, 384:512]
                t1 = stb.tile([128, 128], f32, name="t1", tag="t1")
                stt(t1[0:P104, :], ti, 1.0, tg, op0=ADD, op1=MULT)
                m2 = stb.tile([128, 128], f32, name="m2", tag="m2")
                stt(m2[0:P104, :], tf_, 1.0, cq_sb[0:P104, :], op0=ADD, op1=MULT)
                stt(cq_sb[0:P104, :], m2[0:P104, :], 0.5, t1[0:P104, :],
                    op0=MULT, op1=ADD)
                tc2 = stb.tile([128, 128], bf16, name="tc2", tag="tc2")
                nc.scalar.activation(tc2[0:P104, :], cq_sb[0:P104, :], AF.Tanh,
                                     scale=0.5)
                h_sb = stb.tile([128, 128], bf16, name="h", tag="h")
                stt(h_sb[0:P104, :], tog, 1.0, tc2[0:P104, :], op0=ADD, op1=MULT)

                tp = transpose104(h_sb[0:P104, :])
                hT_sb = stb.tile([128, 4, 32], bf16, name="hT", tag="hT")
                vcp(hT_sb[:, :, 0:BL], tp[:, :, 0:BL])

                # gates(t+1): gy + h-part
                if t < T - 1:
                    gq_next = pg.tile([128, 512], f32, name="gq", tag="gq")
                    for j in range(4):
                        quad_mm(gq_next, id8, gytq[0:8, 512 * j:512 * (j + 1)],
                                j, start=True, stop=False)
                    for k in range(4):
                        for j in range(4):
                            quad_mm(gq_next, hT_sb[:, k, 0:BL],
                                    wr_sb[:, 4 + k, 512 * j:512 * (j + 1)], j,
                                    start=False, stop=False)

                # attention: E (masked) -> exp -> normalize -> transpose
                eq = pe1.tile([128, 2 * R + 1], f32, name="eq", tag="eq")
                for j in range(4):
                    quad_mm(eq, id8, mask_sb[0:8, j, :], j, start=True,
                            stop=False)
                for k in range(4):
                    for j in range(4):
                        mm(eq[32 * j:32 * j + BL, 0:2 * R], hT_sb[:, k, 0:BL],
                           apT_sb[:, k, 2 * R * j:2 * R * (j + 1)],
                           start=False, stop=(k == 3), skip_group_check=True,
                           tile_position=(0, 32 * j))
                expq = stb.tile([128, 2 * R + 1], bf16, name="expq", tag="expq")
                ssum = stb.tile([128, 1], f32, name="ssum", tag="ssum")
                nc.scalar.activation(expq[0:P104, :], eq[0:P104, :], AF.Exp,
                                     accum_out=ssum[0:P104, :])
                rs = stb.tile([128, 1], f32, name="rs", tag="rs")
                nc.vector.reciprocal(rs[0:P104, :], ssum[0:P104, :])
                attq = stb.tile([128, 2 * R], bf16, name="attq", tag="attq")
                nc.vector.tensor_scalar_mul(attq[0:P104, :],
                                            expq[0:P104, 0:2 * R],
                                            rs[0:P104, :])
                tpE = transpose104(attq[0:P104, :])
                vcp(abd_sb[0:2 * R, :, 0:BL], tpE[0:2 * R, :, 0:BL])

                # vo = h @ Wuh + b_u + att-weighted P  (quad)
                vq = pv.tile([128, 128], f32, name="vq", tag="vq")
                for k in range(4):
                    for j in range(4):
                        quad_mm(vq, hT_sb[:, k, 0:BL],
                                wuh_sb[:, k, 128 * j:128 * (j + 1)], j,
                                start=(k == 0), stop=False)
                for m in range(4):
                    kk = 2 * R + 1 if m == 0 else 2 * R
                    for j in range(4):
                        quad_mm(vq, abd_sb[0:kk, m, 0:BL],
                                pbd_sb[0:kk, m, 128 * j:128 * (j + 1)], j,
                                start=False, stop=(m == 3))
                o_sb = stb.tile([128, 128], bf16, name="o", tag="o")
                nc.scalar.activation(o_sb[0:P104, :], vq[0:P104, :], AF.Tanh)

                tpo = transpose104(o_sb[0:P104, :])
                vcp(otb_sb[:, :, t + 1, :], tpo[:, :, 0:BL])

                # gates(t+1): o-part
                if t < T - 1:
                    for k in range(4):
                        for j in range(4):
                            quad_mm(gq_next, otb_sb[:, k, t + 1, :],
                                    wr_sb[:, k, 512 * j:512 * (j + 1)], j,
                                    start=False, stop=(k == 3))
                    gq = gq_next

                # interleave M-tile-0 vocab units into steps 16..31
                if t >= 16:
                    for u in range(24 * (t - 16) // 16, 24 * (t - 15) // 16):
                        vocab_unit(*units[u])

            # ---- vocab M-tile 1 ----
            for u in range(24, 48):
                vocab_unit(*units[u])

    nc.compile()
    return nc


_STATE = {}


def kernel(**inputs):
    from concourse.bass_utils import run_bass_kernel_spmd

    in_maps = prep_inputs(**inputs)
    if "nc" not in _STATE:
        _STATE["nc"] = build_nc()
    nc = _STATE["nc"]
    res = run_bass_kernel_spmd(nc, in_maps, core_ids=list(range(NCORES)))
    bv = np.asarray(inputs["b_vocab"], np.float32)
    full = np.empty((B, T, VOCAB), np.float32)
    for c in range(NCORES):
        o = res.results[c]["out"].astype(np.float32) + bv[None, :]
        full[c * BL:(c + 1) * BL] = o.reshape(T, BL, VOCAB).transpose(1, 0, 2)
    return full
